# revision 36
# baseline (speedup 1.0000x reference)
"""Trainium2 Bass kernel for nn_MixedDecoder (moe_routing).

Math (matches the reference exactly): only the LAST expert layer matters —
the reference never feeds layer outputs back into `z`, so layers 0/1 are
dead code.  Computed per sample b:
    coef = softmax(gate_mlp(z))                        # [B, 8]
    out  = sum_e coef[b,e] * (z @ w2[e]) + coef @ b2   # [B, 256]

Sharding: data-parallel over batch B=2048 across 8 cores (256 rows/core),
weights replicated.  Host-side numpy packs inputs (including a
pre-transposed z) so each core does 8 input DMAs + 1 output DMA and no
on-chip transposes of z.  Matmul inputs are float32r DRAM parameters
(rounded-fp32 PE datapath: bf16 rate at N>=256, ~1.6e-4 scale-relative
error).  ELU is computed as relu(x)+min(exp(x),1) (monotonicity folds the
min into the exp) with the "+1" offset folded into adjusted next-layer
biases.  Expert matmuls keep zT chunks stationary with expert pairs
side-by-side as a [K,512] moving operand; per-expert coefficient scaling
happens on PSUM eviction (per-partition scalar), then a pairwise add tree
split across DVE and GPSIMD.
"""

import numpy as np

N_CORES = 8
B = 2048
IN_SIZE = 288
HIDDEN = 256
E = 8
GATE_H = 64
OUT_SIZE = 256
BL = B // N_CORES          # 256 rows per core
NCH = BL // 128            # 2 batch chunks of 128
KC = 96                    # K chunk size (288 = 3 x 96)
NK = IN_SIZE // KC
W = E * OUT_SIZE           # 2048: one K-chunk's width of packed w2

_CACHE = {}


def _build_nc(reps=1):
    from concourse import bacc
    import concourse.mybir as mybir
    from concourse.tile import TileContext
    from concourse.masks import make_identity

    dt = mybir.dt
    F32 = dt.float32
    F32R = dt.float32r
    AF = mybir.ActivationFunctionType
    OP = mybir.AluOpType
    AX = mybir.AxisListType

    nc = bacc.Bacc("TRN2", target_bir_lowering=False, debug=False)

    # packed inputs (see make_in_maps)
    zT_d = nc.declare_dram_parameter("zTp", [KC, NK * BL], F32R, isOutput=False)
    # gate weights pack: cols 0:192 = g0_w K-chunks; rows 0:64 cols 192:256 =
    # g1_w; rows 0:64 cols 256:264 = g2_w
    GWX = NK * GATE_H + GATE_H + E
    g0w_d = nc.declare_dram_parameter("g0wp", [KC, GWX], F32R, isOutput=False)
    # biases pack: col0 g0_b | col1 b1_adj | col2[0:8] adj2 | row0 cols 3:11 adj2
    sm_d = nc.declare_dram_parameter("smallp", [GATE_H, 12], F32, isOutput=False)
    b2_d = nc.declare_dram_parameter("b2", [E, OUT_SIZE], F32R, isOutput=False)
    # w2 packed: [96, 3*2048]; chunk i cols = w2.transpose(1,0,2)[i*96:(i+1)*96]
    w2_d = nc.declare_dram_parameter("w2p", [KC, NK * W], F32R, isOutput=False)
    out_d = nc.declare_dram_parameter("outp", [128, NCH * OUT_SIZE], F32,
                                      isOutput=True)

    with TileContext(nc) as tc:
      for _rep in range(reps):
        with (
            tc.tile_pool(name="const", bufs=1) as cp,
            tc.tile_pool(name="w2p", bufs=1) as wp,
            tc.tile_pool(name="wk", bufs=2) as wk,
            tc.tile_pool(name="py", bufs=4, space="PSUM") as py,
            tc.tile_pool(name="pb", bufs=2, space="PSUM") as pb,
            tc.tile_pool(name="pg", bufs=2, space="PSUM") as pg,
        ):
            # -------- DMAs: tiny biases, then gate inputs, then w2, b2 ------
            sm = cp.tile([GATE_H, 12], F32, name="sm")
            nc.sync.dma_start(out=sm[:], in_=sm_d.ap())
            zT_r = cp.tile([KC, NK * BL], F32R, name="zT")
            nc.sync.dma_start(out=zT_r[:], in_=zT_d.ap())
            g0w_r = cp.tile([KC, GWX], F32R, name="g0wr")
            nc.sync.dma_start(out=g0w_r[:], in_=g0w_d.ap())
            # w2 packed pair-major: piece p = [96, NK*512] holds expert-pair
            # p's 512 columns for all NK K-chunks, so each Y accumulation
            # group depends on exactly one DMA piece
            PW = NK * 2 * OUT_SIZE      # 1536 cols per pair piece
            w2_r = wp.tile([KC, NK * W], F32R, name="w2r")
            for p in range(E // 2):
                nc.sync.dma_start(out=w2_r[:, p * PW:(p + 1) * PW],
                                  in_=w2_d.ap()[:, p * PW:(p + 1) * PW])
            b2_r = cp.tile([E, OUT_SIZE], F32R, name="b2r")
            nc.sync.dma_start(out=b2_r[:], in_=b2_d.ap())

            ident = cp.tile([128, 128], F32, name="ident")
            make_identity(nc, ident[:])
            ident_r = cp.tile([128, 128], F32R, name="identr")
            nc.vector.tensor_copy(ident_r[:], ident[:])

            # dummy exp so the ACT Exp-table load happens before it's needed
            warm = cp.tile([1, 1], F32, name="warm")
            nc.vector.memset(warm[:], 0.0)
            warm2 = cp.tile([1, 1], F32, name="warm2")
            nc.scalar.activation(warm2[:], warm[:], AF.Exp)

            g1w_r = g0w_r[0:GATE_H, NK * GATE_H:NK * GATE_H + GATE_H]
            g2w_r = g0w_r[0:GATE_H, NK * GATE_H + GATE_H:GWX]
            g0b = sm[:, 0:1]
            b1_adj = sm[:, 1:2]          # g1_b - colsum(g1_w), host-computed
            adj2_col = sm[0:E, 2:3]      # g2_b - colsum(g2_w), host-computed
            adj2_row = sm[0:1, 3:3 + E]
            ones_row = cp.tile([1, 128], F32, name="ones_row")
            nc.vector.memset(ones_row[:], 1.0)

            # PE warm-up: dummy matmuls so the HAM clock-gate releases before
            # the real work arrives (throwaway results)
            wu_ps = pb.tile([128, 128], F32, name="wups", tag="pb")
            for _ in range(8):
                nc.tensor.matmul(wu_ps[:], ident_r[:], ident_r[:],
                                 start=True, stop=True)

            # Gate MLP in transposed layout.  ELU is kept as two pieces
            # (relu(x) and min(exp(x),1), i.e. elu(x)+1 split) and the sum is
            # folded into the NEXT layer's matmul as two PSUM-accumulating
            # matmuls; the "+1" offset is absorbed by host-adjusted biases.
            def elu_pieces(ps_in, bias, pref):
                t_exp = wk.tile([GATE_H, BL], F32, name=f"{pref}_exp")
                nc.scalar.activation(t_exp[:], ps_in, AF.Exp, bias=bias)
                t_min = wk.tile([GATE_H, BL], F32R, name=f"{pref}_min")
                nc.vector.tensor_scalar(t_min[:], t_exp[:], 1.0, None, OP.min)
                t_relu = wk.tile([GATE_H, BL], F32R, name=f"{pref}_relu")
                nc.vector.tensor_scalar(t_relu[:], ps_in, bias, 0.0, OP.add, OP.max)
                return t_relu, t_min

            with tc.high_priority():
                h0_ps = pg.tile([GATE_H, BL], F32, name="h0ps", tag="pg")
                for i in range(NK):
                    nc.tensor.matmul(h0_ps[:], g0w_r[:, i * GATE_H:(i + 1) * GATE_H],
                                     zT_r[:, i * BL:(i + 1) * BL],
                                     start=(i == 0), stop=(i == NK - 1))
                h0_a, h0_b = elu_pieces(h0_ps[:], g0b, "e0")

                h1_ps = pg.tile([GATE_H, BL], F32, name="h1ps", tag="pg")
                nc.tensor.matmul(h1_ps[:], g1w_r, h0_a[:], start=True, stop=False)
                nc.tensor.matmul(h1_ps[:], g1w_r, h0_b[:], start=False, stop=True)
                h1_a, h1_b = elu_pieces(h1_ps[:], b1_adj, "e1")

                # exp(logits) in [b, 8] layout per chunk for per-partition
                # scales (unnormalized; 1/sum is applied at final eviction)
                exp_sb = []    # (expc [128,8], rcp [128,1]) per chunk
                for c in range(NCH):
                    lg_ps = pg.tile([128, E], F32, name="lgps", tag="pg")
                    nc.tensor.matmul(lg_ps[:], h1_a[:, c * 128:(c + 1) * 128],
                                     g2w_r, start=True, stop=False)
                    nc.tensor.matmul(lg_ps[:], h1_b[:, c * 128:(c + 1) * 128],
                                     g2w_r, start=False, stop=False)
                    nc.tensor.matmul(lg_ps[:], ones_row[:], adj2_row,
                                     start=False, stop=True)
                    expc = wk.tile([128, E], F32, name="expc")
                    sume = wk.tile([128, 1], F32, name="sume")
                    nc.scalar.activation(expc[:], lg_ps[:], AF.Exp,
                                         accum_out=sume[:])
                    rcp = wk.tile([128, 1], F32, name="rcp")
                    nc.vector.reciprocal(rcp[:], sume[:])
                    exp_sb.append((expc, rcp))

                # ... and unnormalized exp(logits) in transposed [8, b] layout
                # (only needed later, for the mixed-bias matmul)
                lgT_ps = pg.tile([E, BL], F32, name="lgTps", tag="pg")
                nc.tensor.matmul(lgT_ps[:], g2w_r, h1_a[:], start=True, stop=False)
                nc.tensor.matmul(lgT_ps[:], g2w_r, h1_b[:], start=False, stop=True)
                expT_u = wk.tile([E, BL], F32R, name="expTu")
                nc.scalar.activation(expT_u[:], lgT_ps[:], AF.Exp, bias=adj2_col)

            # ---------------- expert layer + combine ----------------
            # Y_pair matmuls -> coef-scaled fp32r eviction -> PE re-sum:
            # one PSUM accumulation of 8 identity-matmuls + the mixed-bias
            # matmul, so no elementwise add tree is needed.
            out_sb = wk.tile([128, NCH * OUT_SIZE], F32, name="outsb")
            for c in range(NCH):
                ys = []
                for p in range(E // 2):
                    yp = py.tile([128, 2 * OUT_SIZE], F32, name=f"yp{p}", tag="py")
                    for i in range(NK):
                        col0 = p * PW + i * 2 * OUT_SIZE
                        nc.tensor.matmul(
                            yp[:], zT_r[:, i * BL + c * 128:i * BL + (c + 1) * 128],
                            w2_r[:, col0:col0 + 2 * OUT_SIZE],
                            start=(i == 0), stop=(i == NK - 1))
                    for h in range(2):
                        e = 2 * p + h
                        t = wk.tile([128, OUT_SIZE], F32R, name=f"ys{e}")
                        src = yp[:, h * OUT_SIZE:(h + 1) * OUT_SIZE]
                        scale = exp_sb[c][0][:, e:e + 1]
                        if h == 0:
                            nc.scalar.activation(t[:], src, AF.Copy, scale=scale)
                        else:
                            nc.vector.tensor_scalar(t[:], src, scale, None, OP.mult)
                        ys.append(t)

                out_ps = pb.tile([128, OUT_SIZE], F32, name="outps", tag="pb")
                nc.tensor.matmul(out_ps[:], expT_u[:, c * 128:(c + 1) * 128],
                                 b2_r[:], start=True, stop=False)
                for e in range(E):
                    nc.tensor.matmul(out_ps[:], ident_r[:], ys[e][:],
                                     start=False, stop=(e == E - 1))
                # final eviction applies the softmax normalization 1/sum
                nc.scalar.activation(out_sb[:, c * OUT_SIZE:(c + 1) * OUT_SIZE],
                                     out_ps[:], AF.Copy, scale=exp_sb[c][1][:])
                nc.sync.dma_start(
                    out=out_d.ap()[:, c * OUT_SIZE:(c + 1) * OUT_SIZE],
                    in_=out_sb[:, c * OUT_SIZE:(c + 1) * OUT_SIZE])

    nc.finalize()
    return nc


def _get_nc(reps=1):
    key = ("nc", reps)
    if key not in _CACHE:
        _CACHE[key] = _build_nc(reps)
    return _CACHE[key]


def make_in_maps(z, g0_w, g0_b, g1_w, g1_b, g2_w, g2_b, w2, b2, **_unused):
    z = np.asarray(z, dtype=np.float32)
    g0_w = np.asarray(g0_w, dtype=np.float32)
    g1_w = np.asarray(g1_w, dtype=np.float32)
    g2_w = np.asarray(g2_w, dtype=np.float32)
    g0_b = np.asarray(g0_b, dtype=np.float32)
    g1_b = np.asarray(g1_b, dtype=np.float32)
    g2_b = np.asarray(g2_b, dtype=np.float32)
    w2 = np.asarray(w2, dtype=np.float32)
    b2 = np.ascontiguousarray(b2, dtype=np.float32)

    g0wp = np.zeros((KC, NK * GATE_H + GATE_H + E), dtype=np.float32)
    for i in range(NK):
        g0wp[:, i * GATE_H:(i + 1) * GATE_H] = g0_w[i * KC:(i + 1) * KC]
    g0wp[0:GATE_H, NK * GATE_H:NK * GATE_H + GATE_H] = g1_w
    g0wp[0:GATE_H, NK * GATE_H + GATE_H:] = g2_w
    smallp = np.zeros((GATE_H, 12), dtype=np.float32)
    smallp[:, 0] = g0_b
    # adjusted biases absorb the ELU "+1" offset of the previous layer
    smallp[:, 1] = g1_b - g1_w.sum(axis=0)
    adj2 = g2_b - g2_w.sum(axis=0)
    smallp[0:E, 2] = adj2
    smallp[0, 3:3 + E] = adj2
    w2t = np.ascontiguousarray(w2.transpose(1, 0, 2)).reshape(IN_SIZE, W)
    # pair-major packing: piece p = concat over K-chunks of pair p's 512 cols
    w2p = np.concatenate(
        [w2t[i * KC:(i + 1) * KC, 2 * p * OUT_SIZE:(2 * p + 2) * OUT_SIZE]
         for p in range(E // 2) for i in range(NK)], axis=1)

    shared = {
        "g0wp": np.ascontiguousarray(g0wp),
        "smallp": smallp,
        "w2p": np.ascontiguousarray(w2p),
        "b2": b2,
    }
    maps = []
    for c in range(N_CORES):
        zT = z[c * BL:(c + 1) * BL].T                      # [288, 256]
        zTp = np.concatenate([zT[i * KC:(i + 1) * KC] for i in range(NK)],
                             axis=1)                        # [96, 768]
        maps.append(dict(shared, zTp=np.ascontiguousarray(zTp)))
    return maps


def unpack_out(res_list):
    full = np.empty((B, OUT_SIZE), dtype=np.float32)
    for c in range(N_CORES):
        packed = res_list[c]["outp"]
        for ch in range(NCH):
            full[c * BL + ch * 128:c * BL + (ch + 1) * 128] = \
                packed[:, ch * OUT_SIZE:(ch + 1) * OUT_SIZE]
    return full


def kernel(**inputs):
    from concourse.bass_utils import run_bass_kernel_spmd

    nc = _get_nc()
    in_maps = make_in_maps(**inputs)
    res = run_bass_kernel_spmd(nc, in_maps, list(range(N_CORES)))
    return unpack_out(res.results)
